# revision 21
# baseline (speedup 1.0000x reference)
"""Trainium2 Bass kernel for ClinicalStateFormationOperator.

Full-input contract: kernel(**inputs) takes the complete (unsharded) numpy
inputs and returns the full [B, T, V, D] output. Internally the work is
sharded across 8 NeuronCores as (batch, head-group): core c handles batch
c//2 and heads (c%2)*4 .. (c%2)*4+3. Each core computes its 4 heads'
attention and the partial output projection; the host sums the two partial
projections per batch and adds the output bias.

Math notes (per core, N = T*V = 1536 tokens, head_dim=64, obs_dim=16).
Scores are computed transposed (keys on partitions, queries free) in ONE
K=128 matmul per [128k x 512q] tile by packing four contraction groups:

  rows  0: 64  kT_h            |  qT_h * scale         (content; scale is
               folded into Wq on the host -- 1/8 is exact in fp32)
  rows 64: 80  okT_h           |  oqT_h * obs_scale    (observation; both
               sides computed on the host -- a [N,2]@[2,16] expansion, same
               category as the bias gather tables -- and DMA'd into the packs)
  rows 80:112  [K%32==j]       |  VB_h[Q%32, j]        (variable bias)
  rows112:128  A_hj[s,K]=rtb_h[16j+s-K//32+47] | [(Q//32)%16==s]  (time bias;
               the 16-row basis spans the 16 time bins of q-chunk j, so the
               A rows are re-DMA'd into the k-pack once per (head, q-chunk))

  E^T = exp(scores^T)  (no max-subtraction: |scores| <~ 10, fp32-safe)
  [out^T; denom_rep] = [v | ones_64]^T @ E^T  (ones columns replicate the
        softmax denominator across 64 partitions -> aligned divide)
  OT = out^T * reciprocal(denom_rep)
  y_partial = OT^T_heads @ Wo_rows   (host sums core pairs + bo)

All matmuls run in float32r (full-rate fp32 PE mode, ~1e-4 rel err).

Engine budget per core (cost-model cycles): PE ~184k cy @2.4GHz = 76.8us
(projections 37k, scores 74k, attn*V 74k); Act = 48 exp instructions only
(~70us); DVE = q-pack copies + reciprocal + divide (~24us); Pool = k/v/y
copies + ones memsets (~38us). The emission order starts head 0's score
pipeline right after its projections so the Act engine saturates early,
and a short PE warmup loop during the DMA lead-in buys the 2.4GHz p-state
before real work arrives.

q/k/v biases are zero in this problem's setup_inputs; a with-bias variant
(K=1 bias matmuls into the projection psums) is built only if a nonzero
bias is ever passed. boq/bok fold into the host-computed obs rows and bo
is added on the host.
"""

import ml_dtypes
import numpy as np

import concourse.bass as bass
import concourse.mybir as mybir
import concourse.tile as tile
from concourse.bass_utils import run_bass_kernel_spmd

V = 32
T = 48
D = 512
H = 8
HD = D // H          # 64
OD = 16
B = 4
N = T * V            # 1536
HPC = 4              # heads per core
NCORES = 8
SCALE = 1.0 / np.sqrt(HD)      # 1/8, exact in fp32
OBS_SCALE = 1.0 / np.sqrt(OD)  # 1/4, exact in fp32

F32 = mybir.dt.float32
F32R = mybir.dt.float32r
BF16 = mybir.dt.bfloat16
EXP = mybir.ActivationFunctionType.Exp

KC = N // 128        # 12 key chunks of 128
QC = N // 512        # 3 query chunks of 512
GC = KC // 3         # 4 score/exp groups of 3 key chunks per (head, q-chunk)


def _split_waits(nc, max_waits=1):
    """Walrus in this container allows only one sync-wait slot per
    instruction; spill extra waits onto preceding same-engine NoOps."""
    def fix_bb(bb):
        changed = False
        new = []
        for inst in bb.instructions:
            si = inst.sync_info
            if si is not None and len(si.on_wait) > max_waits:
                waits = list(si.on_wait)
                for w in waits[:-max_waits]:
                    new.append(mybir.InstNoOp(
                        name=nc.get_next_instruction_name(),
                        engine=inst.engine, ins=[], outs=[],
                        sync_info=mybir.SyncInfo(on_wait=[w], on_update=[])))
                    changed = True
                si.on_wait = waits[-max_waits:]
            new.append(inst)
        if changed:
            bb.instructions = new
        for sub in getattr(bb, 'blocks', []) or []:
            fix_bb(sub)
    for f in nc.m.functions:
        for bb in f.blocks:
            fix_bb(bb)


def _build(with_bias=False):
    nc = bass.Bass()

    # ---- per-core DRAM I/O (data differs per core, program is SPMD) ----
    fhT = nc.dram_tensor('fhT', [D, N], F32R, kind='ExternalInput')
    wq = nc.dram_tensor('wq', [D, HPC * HD], F32R, kind='ExternalInput')
    wk = nc.dram_tensor('wk', [D, HPC * HD], F32R, kind='ExternalInput')
    wv = nc.dram_tensor('wv', [D, HPC * HD], F32R, kind='ExternalInput')
    wo = nc.dram_tensor('wo', [2, 128, D], BF16, kind='ExternalInput')
    # score-pack static rows (host-built):
    #   qstat[hh] -> q-pack rows 64:128  (oqT*os | VB gather | time one-hot)
    #   kstat[hh] -> k-pack rows 64:112  (okT    | var one-hot)
    #   apack[hh,j] -> k-pack rows 112:128 (time-bias basis per q-chunk)
    qstat = nc.dram_tensor('qstat', [HPC, 64, N], F32R, kind='ExternalInput')
    kstat = nc.dram_tensor('kstat', [HPC, 48, N], F32R, kind='ExternalInput')
    apack = nc.dram_tensor('apack', [HPC, QC, 16, N], F32R,
                           kind='ExternalInput')
    if with_bias:
        bqr = nc.dram_tensor('bqr', [1, HPC * HD], F32R, kind='ExternalInput')
        bkr = nc.dram_tensor('bkr', [1, HPC * HD], F32R, kind='ExternalInput')
        bvr = nc.dram_tensor('bvr', [1, HPC * HD], F32R, kind='ExternalInput')
        onesd = nc.dram_tensor('onesd', [1, 512], F32R, kind='ExternalInput')
    out = nc.dram_tensor('out', [N, D], F32, kind='ExternalOutput')

    with tile.TileContext(nc) as tc:
        with tc.tile_pool(name='cst', bufs=1) as cst, \
             tc.tile_pool(name='big', bufs=1) as big, \
             tc.tile_pool(name='work', bufs=6) as work, \
             tc.tile_pool(name='et', bufs=10) as etp, \
             tc.tile_pool(name='ps3', bufs=2, space='PSUM') as ps3, \
             tc.tile_pool(name='ps', bufs=2, space='PSUM') as ps:

            # ---- PE warmup: keep the tensor engine busy through the DMA
            # lead-in so real matmuls start at the 2.4GHz p-state.
            t_warm = cst.tile([128, 512], BF16)
            nc.gpsimd.memset(t_warm[:], 0.0)
            p_warm = ps.tile([128, 512], F32, tag='mm', name='p_warm')
            for _ in range(13):
                nc.tensor.matmul(p_warm[:], t_warm[:, 0:128], t_warm[:],
                                 start=True, stop=True)

            # ---- tiles ----
            t_wq = cst.tile([128, 4, HPC * HD], F32R)
            t_wk = cst.tile([128, 4, HPC * HD], F32R)
            t_wv = cst.tile([128, 4, HPC * HD], F32R)
            t_wo = cst.tile([128, 2, D], BF16)
            t_fhT = big.tile([128, 4, N], F32R)
            t_qp = [big.tile([128, N], F32R, tag=f'qp{hh}', name=f'qp{hh}')
                    for hh in range(HPC)]
            t_kp = [big.tile([128, N], F32R, tag=f'kp{hh}', name=f'kp{hh}')
                    for hh in range(HPC)]
            t_v = big.tile([128, HPC, KC, 128], BF16, tag='v', name='v')
            t_ot = [big.tile([128, N], BF16, tag=f'ot{p}', name=f'ot{p}')
                    for p in range(2)]

            # ---- input DMAs. Transfers serialize on the DMA engines
            # (~360GB/s aggregate) and HWDGE generation serializes at
            # ~630ns/DMA, so ALL input DMAs go on the SP queue in critical-
            # path order and the Act queue carries exp work only.
            wq_r = wq[:].rearrange('(o p) n -> p o n', p=128)
            wk_r = wk[:].rearrange('(o p) n -> p o n', p=128)
            fhT_r = fhT[:].rearrange('(o p) n -> p o n', p=128)
            nc.sync.dma_start(t_wq[:], wq_r)
            nc.sync.dma_start(t_fhT[:, :, 0:512], fhT_r[:, :, 0:512])
            nc.sync.dma_start(t_wk[:], wk_r)
            nc.sync.dma_start(t_kp[0][64:112, :], kstat[0])
            nc.sync.dma_start(t_kp[0][112:128, :], apack[0, 0])
            nc.sync.dma_start(t_qp[0][64:128, :], qstat[0])
            nc.sync.dma_start(t_fhT[:, :, 512:1024], fhT_r[:, :, 512:1024])
            nc.sync.dma_start(t_wv[:], wv[:].rearrange('(o p) n -> p o n',
                                                       p=128))
            nc.sync.dma_start(t_fhT[:, :, 1024:N], fhT_r[:, :, 1024:N])
            for hh in range(1, HPC):
                nc.sync.dma_start(t_kp[hh][64:112, :], kstat[hh])
                nc.sync.dma_start(t_kp[hh][112:128, :], apack[hh, 0])
                nc.sync.dma_start(t_qp[hh][64:128, :], qstat[hh])
            nc.sync.dma_start(t_wo[:], wo[:].rearrange('o p n -> p o n'))
            if with_bias:
                t_bq = cst.tile([1, HPC * HD], F32R)
                nc.sync.dma_start(t_bq[:], bqr[:])
                t_bk = cst.tile([1, HPC * HD], F32R)
                nc.sync.dma_start(t_bk[:], bkr[:])
                t_bv = cst.tile([1, HPC * HD], F32R)
                nc.sync.dma_start(t_bv[:], bvr[:])
                t_ones = cst.tile([1, 512], F32R)
                nc.sync.dma_start(t_ones[:], onesd[:])

            # ones columns for the softmax-denominator trick
            nc.gpsimd.memset(t_v[:, :, :, 64:128], 1.0)

            # ---- stage 1 emitters ----
            def emit_qk_j(m, j, copy_eng=None, q_first=False):
                # k first by default: the early score groups consume
                # k-content blocks ahead of q. The very first block runs
                # q first since wq lands before wk in the DMA chain.
                order = ((t_wk, 'bk', t_kp), (t_wq, 'bq', t_qp))
                if q_first:
                    order = order[::-1]
                for (w_t, b_name, pack) in order:
                    p_qt = ps.tile([128, 512], F32, tag='mm', name='p_qt')
                    for kk in range(4):
                        nc.tensor.matmul(
                            p_qt[:], w_t[:, kk, m * 128:(m + 1) * 128],
                            t_fhT[:, kk, j * 512:(j + 1) * 512],
                            start=(kk == 0),
                            stop=(not with_bias and kk == 3))
                    if with_bias:
                        bt = t_bq if b_name == 'bq' else t_bk
                        nc.tensor.matmul(
                            p_qt[:], bt[:, m * 128:(m + 1) * 128],
                            t_ones[:], start=False, stop=True)
                    for s in range(2):
                        hh = 2 * m + s
                        if copy_eng == 'act' and b_name == 'bk':
                            # k copies on Act in parallel with q on DVE:
                            # both gate the first score group
                            nc.scalar.copy(
                                pack[hh][0:64, j * 512:(j + 1) * 512],
                                p_qt[s * 64:(s + 1) * 64, :])
                        else:
                            nc.vector.tensor_copy(
                                pack[hh][0:64, j * 512:(j + 1) * 512],
                                p_qt[s * 64:(s + 1) * 64, :])

            def emit_v(kc0, kc1):
                # v natural layout: psum [128 tokens, 256 ch] per token chunk
                for kc in range(kc0, kc1):
                    p_v = ps.tile([128, HPC * HD], F32, tag='mm', name='p_v')
                    for kk in range(4):
                        nc.tensor.matmul(p_v[:],
                                         t_fhT[:, kk, kc * 128:(kc + 1) * 128],
                                         t_wv[:, kk, :], start=(kk == 0),
                                         stop=(not with_bias and kk == 3))
                    if with_bias:
                        nc.tensor.matmul(p_v[:], t_ones[:, 0:128], t_bv[:],
                                         start=False, stop=True)
                    nc.vector.tensor_copy(
                        t_v[:, :, kc, 0:64],
                        p_v[:].rearrange('p (h c) -> p h c', h=HPC))

            def emit_scores(hh, j, g, p_s3):
                # scores/exp outrank stage-1 fillers in the scheduler's
                # ready-heap: the Act engine's exp stream is the critical
                # path, and score groups must land within ~2 exp-slots of
                # their psum ring slot freeing up
                with tc.high_priority(10 ** 6):
                    for i3 in range(3):
                        kc = 3 * g + i3
                        nc.tensor.matmul(
                            p_s3[:, i3, :],
                            t_kp[hh][:, kc * 128:(kc + 1) * 128],
                            t_qp[hh][:, j * 512:(j + 1) * 512],
                            start=True, stop=True)

            def emit_exp(p_s3):
                t_et = etp.tile([128, 3, 512], BF16, tag='et', name='t_et')
                with tc.high_priority(10 ** 6):
                    nc.scalar.activation(t_et[:], p_s3[:], EXP)
                return t_et

            def emit_av(hh, g, t_et, p_ot):
                # middle priority: below scores/exp, above stage-1 fillers,
                # so the drain chain interleaves promptly at the tail
                with tc.high_priority(5 * 10 ** 5):
                    for i3 in range(3):
                        kc = 3 * g + i3
                        nc.tensor.matmul(p_ot[:], t_v[:, hh, kc, :],
                                         t_et[:, i3, :],
                                         start=(kc == 0),
                                         stop=(kc == KC - 1))

            def emit_div(hh, j, p_ot):
                t_rec = work.tile([64, 512], F32, tag='rec', name='t_rec')
                with tc.high_priority(5 * 10 ** 5):
                    nc.vector.reciprocal(t_rec[:], p_ot[64:128, :])
                    nc.vector.tensor_mul(
                        t_ot[hh // 2][(hh % 2) * 64:(hh % 2) * 64 + 64,
                                      j * 512:(j + 1) * 512],
                        p_ot[0:64, :], t_rec[:])

            def emit_yout(j, split=False):
                # partial out-projection for this q-chunk's 4 row blocks.
                # split=True (final block, Act engine idle): alternate the
                # psum copies and output DMAs across engines/queues to
                # shorten the drain tail.
                for qq in range(4):
                    qc = 4 * j + qq
                    p_y = ps.tile([128, D], F32, tag='mm', name='p_y')
                    for p in range(2):
                        nc.tensor.matmul(p_y[:],
                                         t_ot[p][:, qc * 128:(qc + 1) * 128],
                                         t_wo[:, p, :], start=(p == 0),
                                         stop=(p == 1))
                    t_y = work.tile([128, D], F32, tag='y', name='t_y')
                    if split and qq >= 2:
                        nc.scalar.copy(t_y[:], p_y[:])
                    else:
                        nc.vector.tensor_copy(t_y[:], p_y[:])
                    nc.sync.dma_start(out[qc * 128:(qc + 1) * 128, :],
                                      t_y[:])

            # ---- emission: the early phase is paced by the serial DMA
            # chain (fhT is 8.7us of transfer alone), so head 0's score
            # groups are emitted as their q-chunk blocks land, with v /
            # second-head-pair projections interleaved in small slices to
            # fill PE slack without starving the Act engine's exp stream.
            sq = {}

            def sq_scores(g):
                sq[g] = ps3.tile([128, 3, 512], F32, tag='s3', name='p_s3')
                emit_scores(0, 0, g, sq[g])

            emit_qk_j(0, 0, copy_eng='act', q_first=True)
            sq_scores(0)
            emit_qk_j(1, 0)
            emit_qk_j(0, 1)
            sq_scores(1)
            emit_qk_j(0, 2)

            # fillers[(hh, j, g)] emitted right after that group's scores,
            # while its exp runs on Act
            fillers = {
                (0, 0, 0): lambda: emit_qk_j(1, 1),
                (0, 0, 1): lambda: emit_qk_j(1, 2),
                (0, 0, 2): lambda: emit_v(0, 6),
                (0, 0, 3): lambda: emit_v(6, KC),
                (0, 1, 1): lambda: emit_yout(0),
                (0, 2, 1): lambda: emit_yout(1),
            }

            for j in range(QC):
                for hh in range(HPC):
                    # scores + exp (+ stage-1 fillers) first: the filler
                    # projection psums must never coexist with the live
                    # attention accumulator in the 2-buffer 'ps' pool
                    ets = []
                    for g in range(GC):
                        if (hh, j) == (0, 0) and g < 2:
                            p_s3 = sq.pop(g)
                        else:
                            p_s3 = ps3.tile([128, 3, 512], F32, tag='s3',
                                            name='p_s3')
                            emit_scores(hh, j, g, p_s3)
                        ets.append(emit_exp(p_s3))
                        fill = fillers.pop((hh, j, g), None)
                        if fill is not None:
                            fill()
                    p_ot = ps.tile([128, 512], F32, tag='mm', name='p_ot')
                    for g in range(GC):
                        emit_av(hh, g, ets[g], p_ot)
                    if j + 1 < QC:
                        # prefetch next q-chunk's time-bias basis rows; only
                        # WARs this head's just-finished scores, and the next
                        # read is 3 heads (~15us) away
                        nc.sync.dma_start(t_kp[hh][112:128, :],
                                          apack[hh, j + 1])
                    emit_div(hh, j, p_ot)
                if j == QC - 1:
                    emit_yout(j, split=True)

    _split_waits(nc)
    return nc


_NC_CACHE = {}


def _get_nc(with_bias=False):
    if with_bias not in _NC_CACHE:
        _NC_CACHE[with_bias] = _build(with_bias)
    return _NC_CACHE[with_bias]


def _host_prep(h, observation_state, Wq, bq, Wk, bk, Wv, bv, Wo, bo,
               Woq, boq, Wok, bok, variable_bias, relative_time_bias,
               with_bias=False):
    f32 = np.float32
    h = np.asarray(h, f32)
    obs = np.asarray(observation_state, f32).reshape(B, N, 2)
    Kidx = np.arange(N)
    tK = Kidx // V                                     # time bin of each token
    varsel = (Kidx[None, :] % V == np.arange(V)[:, None]).astype(f32)
    timesel = ((Kidx[None, :] // V) % 16 == np.arange(16)[:, None]).astype(f32)

    # full observation projections (tiny: [B,N,2] @ [2,128])
    oqT = (obs @ np.asarray(Woq, f32) + np.asarray(boq, f32)) * OBS_SCALE
    okT = obs @ np.asarray(Wok, f32) + np.asarray(bok, f32)
    oqT = oqT.transpose(0, 2, 1)                       # [B, H*OD, N]
    okT = okT.transpose(0, 2, 1)

    in_maps = []
    for c in range(NCORES):
        b, hg = divmod(c, 2)
        h0 = hg * HPC
        cs, ce = h0 * HD, (h0 + HPC) * HD
        qstat = np.empty((HPC, 64, N), f32)
        kst = np.empty((HPC, 48, N), f32)
        ap = np.empty((HPC, QC, 16, N), f32)
        for hh in range(HPC):
            head = h0 + hh
            vb = np.asarray(variable_bias[head], f32)
            rtb = np.asarray(relative_time_bias[head], f32)
            qstat[hh, 0:16] = oqT[b, head * OD:(head + 1) * OD]
            qstat[hh, 16:48] = vb[Kidx % V, :].T       # VB_h[Q%32, j]
            qstat[hh, 48:64] = timesel
            kst[hh, 0:16] = okT[b, head * OD:(head + 1) * OD]
            kst[hh, 16:48] = varsel
            for j in range(QC):
                # A_hj[s, K] = rtb[16j + s - K//32 + 47]
                idx = 16 * j + np.arange(16)[:, None] - tK[None, :] + (T - 1)
                ap[hh, j] = rtb[idx]
        m = {
            'fhT': np.ascontiguousarray(h[b].reshape(N, D).T),
            'wq': np.ascontiguousarray(np.asarray(Wq, f32)[:, cs:ce]) * SCALE,
            'wk': np.ascontiguousarray(np.asarray(Wk, f32)[:, cs:ce]),
            'wv': np.ascontiguousarray(np.asarray(Wv, f32)[:, cs:ce]),
            'wo': np.ascontiguousarray(
                np.asarray(Wo, f32)[cs:ce, :].reshape(2, 128, D)).astype(
                    ml_dtypes.bfloat16),
            'qstat': qstat,
            'kstat': kst,
            'apack': ap,
        }
        if with_bias:
            m.update({
                'bqr': np.asarray(bq, f32)[None, cs:ce] * SCALE,
                'bkr': np.ascontiguousarray(np.asarray(bk, f32)[None, cs:ce]),
                'bvr': np.ascontiguousarray(np.asarray(bv, f32)[None, cs:ce]),
                'onesd': np.ones((1, 512), f32),
            })
        in_maps.append(m)
    return in_maps


def kernel(**inputs):
    with_bias = any(
        np.any(np.asarray(inputs[k])) for k in ('bq', 'bk', 'bv'))
    nc = _get_nc(with_bias)
    in_maps = _host_prep(**inputs, with_bias=with_bias)
    res = run_bass_kernel_spmd(nc, in_maps, core_ids=list(range(NCORES)))
    bo = np.asarray(inputs['bo'], np.float32)
    outf = np.zeros((B, N, D), np.float32)
    for c in range(NCORES):
        outf[c // 2] += res.results[c]['out']
    outf += bo[None, None, :]
    return outf.reshape(B, T, V, D)


# revision 22
# speedup vs baseline: 1.0074x; 1.0074x over previous
"""Trainium2 Bass kernel for ClinicalStateFormationOperator.

Full-input contract: kernel(**inputs) takes the complete (unsharded) numpy
inputs and returns the full [B, T, V, D] output. Internally the work is
sharded across 8 NeuronCores as (batch, head-group): core c handles batch
c//2 and heads (c%2)*4 .. (c%2)*4+3. Each core computes its 4 heads'
attention and the partial output projection; the host sums the two partial
projections per batch and adds the output bias.

Math notes (per core, N = T*V = 1536 tokens, head_dim=64, obs_dim=16).
Scores are computed transposed (keys on partitions, queries free) in ONE
K=128 matmul per [128k x 512q] tile by packing four contraction groups:

  rows  0: 64  kT_h            |  qT_h * scale         (content; scale is
               folded into Wq on the host -- 1/8 is exact in fp32)
  rows 64: 80  okT_h           |  oqT_h * obs_scale    (observation; both
               sides computed on the host -- a [N,2]@[2,16] expansion, same
               category as the bias gather tables -- and DMA'd into the packs)
  rows 80:112  [K%32==j]       |  VB_h[Q%32, j]        (variable bias)
  rows112:128  A_hj[s,K]=rtb_h[16j+s-K//32+47] | [(Q//32)%16==s]  (time bias;
               the 16-row basis spans the 16 time bins of q-chunk j, so the
               A rows are re-DMA'd into the k-pack once per (head, q-chunk))

  E^T = exp(scores^T)  (no max-subtraction: |scores| <~ 10, fp32-safe)
  [out^T; denom_rep] = [v | ones_64]^T @ E^T  (ones columns replicate the
        softmax denominator across 64 partitions -> aligned divide)
  OT = out^T * reciprocal(denom_rep)
  y_partial = OT^T_heads @ Wo_rows   (host sums core pairs + bo)

All matmuls run in float32r (full-rate fp32 PE mode, ~1e-4 rel err).

Engine budget per core (cost-model cycles): PE ~184k cy @2.4GHz = 76.8us
(projections 37k, scores 74k, attn*V 74k); Act = 48 exp instructions only
(~70us); DVE = q-pack copies + reciprocal + divide (~24us); Pool = k/v/y
copies + ones memsets (~38us). The emission order starts head 0's score
pipeline right after its projections so the Act engine saturates early,
and a short PE warmup loop during the DMA lead-in buys the 2.4GHz p-state
before real work arrives.

q/k/v biases are zero in this problem's setup_inputs; a with-bias variant
(K=1 bias matmuls into the projection psums) is built only if a nonzero
bias is ever passed. boq/bok fold into the host-computed obs rows and bo
is added on the host.
"""

import ml_dtypes
import numpy as np

import concourse.bass as bass
import concourse.mybir as mybir
import concourse.tile as tile
from concourse.bass_utils import run_bass_kernel_spmd

V = 32
T = 48
D = 512
H = 8
HD = D // H          # 64
OD = 16
B = 4
N = T * V            # 1536
HPC = 4              # heads per core
NCORES = 8
SCALE = 1.0 / np.sqrt(HD)      # 1/8, exact in fp32
OBS_SCALE = 1.0 / np.sqrt(OD)  # 1/4, exact in fp32

F32 = mybir.dt.float32
F32R = mybir.dt.float32r
BF16 = mybir.dt.bfloat16
EXP = mybir.ActivationFunctionType.Exp

KC = N // 128        # 12 key chunks of 128
QC = N // 512        # 3 query chunks of 512
GC = KC // 3         # 4 score/exp groups of 3 key chunks per (head, q-chunk)


def _split_waits(nc, max_waits=1):
    """Walrus in this container allows only one sync-wait slot per
    instruction; spill extra waits onto preceding same-engine NoOps."""
    def fix_bb(bb):
        changed = False
        new = []
        for inst in bb.instructions:
            si = inst.sync_info
            if si is not None and len(si.on_wait) > max_waits:
                waits = list(si.on_wait)
                for w in waits[:-max_waits]:
                    new.append(mybir.InstNoOp(
                        name=nc.get_next_instruction_name(),
                        engine=inst.engine, ins=[], outs=[],
                        sync_info=mybir.SyncInfo(on_wait=[w], on_update=[])))
                    changed = True
                si.on_wait = waits[-max_waits:]
            new.append(inst)
        if changed:
            bb.instructions = new
        for sub in getattr(bb, 'blocks', []) or []:
            fix_bb(sub)
    for f in nc.m.functions:
        for bb in f.blocks:
            fix_bb(bb)


def _build(with_bias=False):
    nc = bass.Bass()

    # ---- per-core DRAM I/O (data differs per core, program is SPMD) ----
    fhT = nc.dram_tensor('fhT', [D, N], F32R, kind='ExternalInput')
    wq = nc.dram_tensor('wq', [D, HPC * HD], F32R, kind='ExternalInput')
    wk = nc.dram_tensor('wk', [D, HPC * HD], F32R, kind='ExternalInput')
    wv = nc.dram_tensor('wv', [D, HPC * HD], F32R, kind='ExternalInput')
    wo = nc.dram_tensor('wo', [2, 128, D], BF16, kind='ExternalInput')
    # score-pack static rows (host-built):
    #   qstat[hh] -> q-pack rows 64:128  (oqT*os | VB gather | time one-hot)
    #   kstat[hh] -> k-pack rows 64:112  (okT    | var one-hot)
    #   apack[hh,j] -> k-pack rows 112:128 (time-bias basis per q-chunk)
    qstat = nc.dram_tensor('qstat', [HPC, 64, N], F32R, kind='ExternalInput')
    kstat = nc.dram_tensor('kstat', [HPC, 48, N], F32R, kind='ExternalInput')
    apack = nc.dram_tensor('apack', [HPC, QC, 16, N], F32R,
                           kind='ExternalInput')
    if with_bias:
        bqr = nc.dram_tensor('bqr', [1, HPC * HD], F32R, kind='ExternalInput')
        bkr = nc.dram_tensor('bkr', [1, HPC * HD], F32R, kind='ExternalInput')
        bvr = nc.dram_tensor('bvr', [1, HPC * HD], F32R, kind='ExternalInput')
        onesd = nc.dram_tensor('onesd', [1, 512], F32R, kind='ExternalInput')
    out = nc.dram_tensor('out', [N, D], F32, kind='ExternalOutput')

    with tile.TileContext(nc) as tc:
        with tc.tile_pool(name='cst', bufs=1) as cst, \
             tc.tile_pool(name='big', bufs=1) as big, \
             tc.tile_pool(name='work', bufs=6) as work, \
             tc.tile_pool(name='et', bufs=10) as etp, \
             tc.tile_pool(name='ps3', bufs=2, space='PSUM') as ps3, \
             tc.tile_pool(name='ps', bufs=2, space='PSUM') as ps:

            # ---- PE warmup: keep the tensor engine busy through the DMA
            # lead-in so real matmuls start at the 2.4GHz p-state.
            t_warm = cst.tile([128, 512], BF16)
            nc.gpsimd.memset(t_warm[:], 0.0)
            p_warm = ps.tile([128, 512], F32, tag='mm', name='p_warm')
            for _ in range(13):
                nc.tensor.matmul(p_warm[:], t_warm[:, 0:128], t_warm[:],
                                 start=True, stop=True)

            # ---- tiles ----
            t_wq = cst.tile([128, 4, HPC * HD], F32R)
            t_wk = cst.tile([128, 4, HPC * HD], F32R)
            t_wv = cst.tile([128, 4, HPC * HD], F32R)
            t_wo = cst.tile([128, 2, D], BF16)
            t_fhT = big.tile([128, 4, N], F32R)
            t_qp = [big.tile([128, N], F32R, tag=f'qp{hh}', name=f'qp{hh}')
                    for hh in range(HPC)]
            t_kp = [big.tile([128, N], F32R, tag=f'kp{hh}', name=f'kp{hh}')
                    for hh in range(HPC)]
            t_v = big.tile([128, HPC, KC, 128], BF16, tag='v', name='v')
            t_ot = [big.tile([128, N], BF16, tag=f'ot{p}', name=f'ot{p}')
                    for p in range(2)]

            # ---- input DMAs. Transfers serialize on the DMA engines
            # (~360GB/s aggregate) and HWDGE generation serializes at
            # ~630ns/DMA, so ALL input DMAs go on the SP queue in critical-
            # path order and the Act queue carries exp work only.
            wq_r = wq[:].rearrange('(o p) n -> p o n', p=128)
            wk_r = wk[:].rearrange('(o p) n -> p o n', p=128)
            fhT_r = fhT[:].rearrange('(o p) n -> p o n', p=128)
            nc.sync.dma_start(t_wq[:], wq_r)
            nc.sync.dma_start(t_fhT[:, :, 0:512], fhT_r[:, :, 0:512])
            nc.sync.dma_start(t_wk[:], wk_r)
            nc.sync.dma_start(t_kp[0][64:112, :], kstat[0])
            nc.sync.dma_start(t_kp[0][112:128, :], apack[0, 0])
            nc.sync.dma_start(t_qp[0][64:128, :], qstat[0])
            nc.sync.dma_start(t_fhT[:, :, 512:1024], fhT_r[:, :, 512:1024])
            nc.sync.dma_start(t_wv[:], wv[:].rearrange('(o p) n -> p o n',
                                                       p=128))
            nc.sync.dma_start(t_fhT[:, :, 1024:N], fhT_r[:, :, 1024:N])
            for hh in range(1, HPC):
                nc.sync.dma_start(t_kp[hh][64:112, :], kstat[hh])
                nc.sync.dma_start(t_kp[hh][112:128, :], apack[hh, 0])
                nc.sync.dma_start(t_qp[hh][64:128, :], qstat[hh])
            nc.sync.dma_start(t_wo[:], wo[:].rearrange('o p n -> p o n'))
            if with_bias:
                t_bq = cst.tile([1, HPC * HD], F32R)
                nc.sync.dma_start(t_bq[:], bqr[:])
                t_bk = cst.tile([1, HPC * HD], F32R)
                nc.sync.dma_start(t_bk[:], bkr[:])
                t_bv = cst.tile([1, HPC * HD], F32R)
                nc.sync.dma_start(t_bv[:], bvr[:])
                t_ones = cst.tile([1, 512], F32R)
                nc.sync.dma_start(t_ones[:], onesd[:])

            # ones columns for the softmax-denominator trick
            nc.gpsimd.memset(t_v[:, :, :, 64:128], 1.0)

            # ---- stage 1 emitters ----
            def emit_qk_j(m, j, copy_eng=None, q_first=False):
                # k first by default: the early score groups consume
                # k-content blocks ahead of q. The very first block runs
                # q first since wq lands before wk in the DMA chain.
                order = ((t_wk, 'bk', t_kp), (t_wq, 'bq', t_qp))
                if q_first:
                    order = order[::-1]
                for (w_t, b_name, pack) in order:
                    p_qt = ps.tile([128, 512], F32, tag='mm', name='p_qt')
                    for kk in range(4):
                        nc.tensor.matmul(
                            p_qt[:], w_t[:, kk, m * 128:(m + 1) * 128],
                            t_fhT[:, kk, j * 512:(j + 1) * 512],
                            start=(kk == 0),
                            stop=(not with_bias and kk == 3))
                    if with_bias:
                        bt = t_bq if b_name == 'bq' else t_bk
                        nc.tensor.matmul(
                            p_qt[:], bt[:, m * 128:(m + 1) * 128],
                            t_ones[:], start=False, stop=True)
                    for s in range(2):
                        hh = 2 * m + s
                        if copy_eng == 'act' and b_name == 'bk':
                            # k copies on Act in parallel with q on DVE:
                            # both gate the first score group
                            nc.scalar.copy(
                                pack[hh][0:64, j * 512:(j + 1) * 512],
                                p_qt[s * 64:(s + 1) * 64, :])
                        else:
                            nc.vector.tensor_copy(
                                pack[hh][0:64, j * 512:(j + 1) * 512],
                                p_qt[s * 64:(s + 1) * 64, :])

            def emit_v(kc0, kc1):
                # v natural layout: psum [128 tokens, 256 ch] per token chunk
                for kc in range(kc0, kc1):
                    p_v = ps.tile([128, HPC * HD], F32, tag='mm', name='p_v')
                    for kk in range(4):
                        nc.tensor.matmul(p_v[:],
                                         t_fhT[:, kk, kc * 128:(kc + 1) * 128],
                                         t_wv[:, kk, :], start=(kk == 0),
                                         stop=(not with_bias and kk == 3))
                    if with_bias:
                        nc.tensor.matmul(p_v[:], t_ones[:, 0:128], t_bv[:],
                                         start=False, stop=True)
                    nc.vector.tensor_copy(
                        t_v[:, :, kc, 0:64],
                        p_v[:].rearrange('p (h c) -> p h c', h=HPC))

            def emit_scores(hh, j, g, p_s3):
                # scores/exp outrank stage-1 fillers in the scheduler's
                # ready-heap: the Act engine's exp stream is the critical
                # path, and score groups must land within ~2 exp-slots of
                # their psum ring slot freeing up
                with tc.high_priority(10 ** 6):
                    for i3 in range(3):
                        kc = 3 * g + i3
                        nc.tensor.matmul(
                            p_s3[:, i3, :],
                            t_kp[hh][:, kc * 128:(kc + 1) * 128],
                            t_qp[hh][:, j * 512:(j + 1) * 512],
                            start=True, stop=True)

            def emit_exp(p_s3):
                t_et = etp.tile([128, 3, 512], BF16, tag='et', name='t_et')
                with tc.high_priority(10 ** 6):
                    nc.scalar.activation(t_et[:], p_s3[:], EXP)
                return t_et

            def emit_av(hh, g, t_et, p_ot, prio=None):
                ctx = tc.high_priority(prio) if prio else None
                if ctx:
                    ctx.__enter__()
                for i3 in range(3):
                    kc = 3 * g + i3
                    nc.tensor.matmul(p_ot[:], t_v[:, hh, kc, :],
                                     t_et[:, i3, :],
                                     start=(kc == 0),
                                     stop=(kc == KC - 1))
                if ctx:
                    ctx.__exit__(None, None, None)

            def emit_div(hh, j, p_ot):
                t_rec = work.tile([64, 512], F32, tag='rec', name='t_rec')
                with tc.high_priority(5 * 10 ** 5):
                    nc.vector.reciprocal(t_rec[:], p_ot[64:128, :])
                    nc.vector.tensor_mul(
                        t_ot[hh // 2][(hh % 2) * 64:(hh % 2) * 64 + 64,
                                      j * 512:(j + 1) * 512],
                        p_ot[0:64, :], t_rec[:])

            def emit_yout(j, split=False):
                # partial out-projection for this q-chunk's 4 row blocks.
                # split=True (final block, Act engine idle): alternate the
                # psum copies and output DMAs across engines/queues to
                # shorten the drain tail.
                for qq in range(4):
                    qc = 4 * j + qq
                    p_y = ps.tile([128, D], F32, tag='mm', name='p_y')
                    for p in range(2):
                        nc.tensor.matmul(p_y[:],
                                         t_ot[p][:, qc * 128:(qc + 1) * 128],
                                         t_wo[:, p, :], start=(p == 0),
                                         stop=(p == 1))
                    t_y = work.tile([128, D], F32, tag='y', name='t_y')
                    if split and qq >= 2:
                        nc.scalar.copy(t_y[:], p_y[:])
                    else:
                        nc.vector.tensor_copy(t_y[:], p_y[:])
                    nc.sync.dma_start(out[qc * 128:(qc + 1) * 128, :],
                                      t_y[:])

            # ---- emission: the early phase is paced by the serial DMA
            # chain (fhT is 8.7us of transfer alone), so head 0's score
            # groups are emitted as their q-chunk blocks land, with v /
            # second-head-pair projections interleaved in small slices to
            # fill PE slack without starving the Act engine's exp stream.
            sq = {}

            def sq_scores(g):
                sq[g] = ps3.tile([128, 3, 512], F32, tag='s3', name='p_s3')
                emit_scores(0, 0, g, sq[g])

            emit_qk_j(0, 0, copy_eng='act', q_first=True)
            sq_scores(0)
            emit_qk_j(1, 0)
            emit_qk_j(0, 1)
            sq_scores(1)
            emit_qk_j(0, 2)

            # fillers[(hh, j, g)] emitted right after that group's scores,
            # while its exp runs on Act
            fillers = {
                (0, 0, 0): lambda: emit_qk_j(1, 1),
                (0, 0, 1): lambda: emit_qk_j(1, 2),
                (0, 0, 2): lambda: emit_v(0, 6),
                (0, 0, 3): lambda: emit_v(6, KC),
                (0, 1, 1): lambda: emit_yout(0),
                (0, 2, 1): lambda: emit_yout(1),
            }

            for j in range(QC):
                for hh in range(HPC):
                    # scores + exp (+ stage-1 fillers) first: the filler
                    # projection psums must never coexist with the live
                    # attention accumulator in the 2-buffer 'ps' pool
                    ets = []
                    for g in range(GC):
                        if (hh, j) == (0, 0) and g < 2:
                            p_s3 = sq.pop(g)
                        else:
                            p_s3 = ps3.tile([128, 3, 512], F32, tag='s3',
                                            name='p_s3')
                            emit_scores(hh, j, g, p_s3)
                        ets.append(emit_exp(p_s3))
                        fill = fillers.pop((hh, j, g), None)
                        if fill is not None:
                            fill()
                    p_ot = ps.tile([128, 512], F32, tag='mm', name='p_ot')
                    last = (j == QC - 1 and hh == HPC - 1)
                    for g in range(GC):
                        emit_av(hh, g, ets[g], p_ot,
                                prio=5 * 10 ** 5 if last else None)
                    if j + 1 < QC:
                        # prefetch next q-chunk's time-bias basis rows; only
                        # WARs this head's just-finished scores, and the next
                        # read is 3 heads (~15us) away
                        nc.sync.dma_start(t_kp[hh][112:128, :],
                                          apack[hh, j + 1])
                    emit_div(hh, j, p_ot)
                if j == QC - 1:
                    emit_yout(j, split=True)

    _split_waits(nc)
    return nc


_NC_CACHE = {}


def _get_nc(with_bias=False):
    if with_bias not in _NC_CACHE:
        _NC_CACHE[with_bias] = _build(with_bias)
    return _NC_CACHE[with_bias]


def _host_prep(h, observation_state, Wq, bq, Wk, bk, Wv, bv, Wo, bo,
               Woq, boq, Wok, bok, variable_bias, relative_time_bias,
               with_bias=False):
    f32 = np.float32
    h = np.asarray(h, f32)
    obs = np.asarray(observation_state, f32).reshape(B, N, 2)
    Kidx = np.arange(N)
    tK = Kidx // V                                     # time bin of each token
    varsel = (Kidx[None, :] % V == np.arange(V)[:, None]).astype(f32)
    timesel = ((Kidx[None, :] // V) % 16 == np.arange(16)[:, None]).astype(f32)

    # full observation projections (tiny: [B,N,2] @ [2,128])
    oqT = (obs @ np.asarray(Woq, f32) + np.asarray(boq, f32)) * OBS_SCALE
    okT = obs @ np.asarray(Wok, f32) + np.asarray(bok, f32)
    oqT = oqT.transpose(0, 2, 1)                       # [B, H*OD, N]
    okT = okT.transpose(0, 2, 1)

    in_maps = []
    for c in range(NCORES):
        b, hg = divmod(c, 2)
        h0 = hg * HPC
        cs, ce = h0 * HD, (h0 + HPC) * HD
        qstat = np.empty((HPC, 64, N), f32)
        kst = np.empty((HPC, 48, N), f32)
        ap = np.empty((HPC, QC, 16, N), f32)
        for hh in range(HPC):
            head = h0 + hh
            vb = np.asarray(variable_bias[head], f32)
            rtb = np.asarray(relative_time_bias[head], f32)
            qstat[hh, 0:16] = oqT[b, head * OD:(head + 1) * OD]
            qstat[hh, 16:48] = vb[Kidx % V, :].T       # VB_h[Q%32, j]
            qstat[hh, 48:64] = timesel
            kst[hh, 0:16] = okT[b, head * OD:(head + 1) * OD]
            kst[hh, 16:48] = varsel
            for j in range(QC):
                # A_hj[s, K] = rtb[16j + s - K//32 + 47]
                idx = 16 * j + np.arange(16)[:, None] - tK[None, :] + (T - 1)
                ap[hh, j] = rtb[idx]
        m = {
            'fhT': np.ascontiguousarray(h[b].reshape(N, D).T),
            'wq': np.ascontiguousarray(np.asarray(Wq, f32)[:, cs:ce]) * SCALE,
            'wk': np.ascontiguousarray(np.asarray(Wk, f32)[:, cs:ce]),
            'wv': np.ascontiguousarray(np.asarray(Wv, f32)[:, cs:ce]),
            'wo': np.ascontiguousarray(
                np.asarray(Wo, f32)[cs:ce, :].reshape(2, 128, D)).astype(
                    ml_dtypes.bfloat16),
            'qstat': qstat,
            'kstat': kst,
            'apack': ap,
        }
        if with_bias:
            m.update({
                'bqr': np.asarray(bq, f32)[None, cs:ce] * SCALE,
                'bkr': np.ascontiguousarray(np.asarray(bk, f32)[None, cs:ce]),
                'bvr': np.ascontiguousarray(np.asarray(bv, f32)[None, cs:ce]),
                'onesd': np.ones((1, 512), f32),
            })
        in_maps.append(m)
    return in_maps


def kernel(**inputs):
    with_bias = any(
        np.any(np.asarray(inputs[k])) for k in ('bq', 'bk', 'bv'))
    nc = _get_nc(with_bias)
    in_maps = _host_prep(**inputs, with_bias=with_bias)
    res = run_bass_kernel_spmd(nc, in_maps, core_ids=list(range(NCORES)))
    bo = np.asarray(inputs['bo'], np.float32)
    outf = np.zeros((B, N, D), np.float32)
    for c in range(NCORES):
        outf[c // 2] += res.results[c]['out']
    outf += bo[None, None, :]
    return outf.reshape(B, T, V, D)
